# revision 14
# baseline (speedup 1.0000x reference)
"""Per-sample dynamic conv2d (VALID) on 8 Trainium2 NeuronCores — v4.

v4 = 1-D Winograd F(2,3) along W, direct 3-tap accumulation along H.
Cuts the PE moving-column count 1.5x vs the direct v3 kernel (145k ->
97k streamed columns per sample).

Math per sample (hp = output row, jt = W-tile, w = 2*jt + a):
  V0 = x[2j]-x[2j+2], V1 = x[2j+1]+x[2j+2],
  V2 = x[2j+2]-x[2j+1], V3 = x[2j+1]-x[2j+3]          (host, bf16)
  U[kh,xi] = G @ K[kh,:]  (G = F(2,3) kernel transform) (host, bf16)
  M[xi][hp] = sum_kh V[xi][hp+kh] @ U[kh,xi]            (PE, 12 MMs/group)
  y_even = M0+M1+M2,  y_odd = M1-M2-M3                  (ACT copies + DVE adds)

Device layout: psum/output partitions = Cout.  Output is written as
[C, 2, HO, 63] (even/odd de-interleaved) and the host transposes back —
all device DMA is therefore fully linear.

Per 8-row group x 4 xi: 3 accumulating matmuls (stationary U[kh,xi],
moving = contiguous 504-col V slice) into 4 psum banks; ACT evacuates
M0..M2 to bf16 SBUF, DVE evacuates M3 and does the 4 inverse-transform
adds.  2 groups of psum (8 banks) in flight.
"""

import numpy as np
import ml_dtypes

import concourse.bass as bass
import concourse.mybir as mybir
from concourse.bass_utils import run_bass_kernel_spmd
from concourse.tile import TileContext

N_CORES = 8
B, H, W, C = 32, 128, 128, 128
KK = 3
XI = 4                       # Winograd phases
BL = B // N_CORES            # samples per core
HO = WO = H - KK + 1         # 126
NJ = WO // 2                 # 63 W-tiles
HPG = 8                      # output rows per group
NG = (HO + HPG - 1) // HPG   # 16 groups (last holds 6 rows)
VSZ = XI * H * NJ            # 32256 free elems of V per sample

F32 = mybir.dt.float32
BF16 = mybir.dt.bfloat16


def _split_excess_waits(nc, limit=1):
    """walrus codegen rejects >1 sync-wait on several instruction kinds.
    Move excess waits onto preceding same-engine NoOps."""
    n = 0
    for bb in nc.m.functions[0].blocks:
        out = []
        changed = False
        for inst in bb.instructions:
            si = inst.sync_info
            if si is not None and len(si.on_wait) > limit:
                waits = list(si.on_wait)
                excess, keep = waits[:-limit], waits[-limit:]
                for i in range(0, len(excess), limit):
                    n += 1
                    out.append(
                        mybir.InstNoOp(
                            name=f"I-waitsplit-{n}",
                            engine=inst.engine,
                            bass_nofuse=True,
                            sync_info=mybir.SyncInfo(
                                on_wait=excess[i : i + limit], on_update=[]
                            ),
                        )
                    )
                inst.sync_info = mybir.SyncInfo(on_wait=keep, on_update=si.on_update)
                changed = True
            out.append(inst)
        if changed:
            bb.instructions = out
    return n


RT = 4                       # V row-tiles per sample
RTR = 34                     # rows per V tile (32 + 2 overlap; last tile 32)


def _build():
    nc = bass.Bass()
    # r-major V so early row-tiles arrive first and group 0 starts ~6us in
    Vd = nc.declare_dram_parameter("V", [BL, C, H, XI, NJ], BF16, isOutput=False)
    # t = kh*4 + xi
    Ud = nc.declare_dram_parameter("U", [BL, KK * XI, C, C], BF16, isOutput=False)
    Od = nc.declare_dram_parameter("out", [BL, C, 2, HO, NJ], BF16, isOutput=True)

    with TileContext(nc) as tc:
        with (
            tc.tile_pool(name="vt", bufs=8) as p_v,
            tc.tile_pool(name="ut", bufs=2) as p_u,
            tc.tile_pool(name="mt", bufs=12) as p_m,
            tc.tile_pool(name="tt", bufs=6) as p_t,
            tc.tile_pool(name="yt", bufs=6) as p_y,
            tc.tile_pool(name="pacc", bufs=4, space="PSUM") as p_acc,
        ):
            def emit_load(b):
                vts = []
                for k in range(RT):
                    r0 = 32 * k
                    nr = min(RTR, H - r0)
                    vt = p_v.tile(
                        [C, RTR * XI * NJ], BF16, tag="vt", name=f"vt{b}_{k}"
                    )
                    nc.sync.dma_start(
                        out=vt[:, 0 : nr * XI * NJ],
                        in_=Vd[b, :, r0 : r0 + nr].rearrange(
                            "c r x j -> c (r x j)"
                        ),
                    )
                    vts.append(vt)
                ut = p_u.tile([C, KK * XI * C], BF16, tag="ut")
                nc.gpsimd.dma_start(
                    out=ut[:, :].rearrange("ci (t co) -> ci t co", t=KK * XI),
                    in_=Ud[b].rearrange("t ci co -> ci t co"),
                )
                return (vts, ut)

            def emit_compute(b, st):
                vts, ut = st
                for g in range(NG):
                    hp0 = HPG * g
                    nh = min(HPG, HO - hp0)
                    n = nh * NJ
                    vt = vts[g // 4]
                    lr0 = hp0 - 32 * (g // 4)  # local row of hp0 in its tile
                    # Pair psum banks: P01 holds M0 at [:,0:n] (bank A) and
                    # M1 at [:,512:512+n] (bank B); P23 likewise. ACT then
                    # evacuates each pair in ONE activate (FD=512+n).
                    P01 = p_acc.tile([C, 1024], F32, tag="P", name=f"P01_{g}")
                    P23 = p_acc.tile([C, 1024], F32, tag="P", name=f"P23_{g}")
                    for xi in range(XI):
                        pt = P01 if xi < 2 else P23
                        o0 = (xi % 2) * 512
                        for kh in range(KK):
                            lr = lr0 + kh
                            nc.tensor.matmul(
                                pt[:, o0 : o0 + n],
                                ut[:, (kh * XI + xi) * C : (kh * XI + xi + 1) * C],
                                vt.rearrange(
                                    "c (r x j) -> c r x j", x=XI, j=NJ
                                )[:, lr : lr + nh, xi, :],
                                start=(kh == 0),
                                stop=(kh == KK - 1),
                            )
                    # ACT evacuates both psum pairs (bf16); DVE does only the
                    # inverse-transform adds (bf16 SBUF, 2x DVE mode).
                    m01 = p_m.tile([C, 1024], BF16, tag="m", name=f"m01_{g}")
                    m23 = p_m.tile([C, 1024], BF16, tag="m", name=f"m23_{g}")
                    nc.scalar.copy(m01[:, 0 : 512 + n], P01[:, 0 : 512 + n])
                    nc.scalar.copy(m23[:, 0 : 512 + n], P23[:, 0 : 512 + n])
                    m0, m1 = m01[:, 0:n], m01[:, 512 : 512 + n]
                    m2, m3 = m23[:, 0:n], m23[:, 512 : 512 + n]
                    te = p_t.tile([C, 504], BF16, tag="t")
                    to = p_t.tile([C, 504], BF16, tag="t")
                    yt = p_y.tile([C, 2 * 504], BF16, tag="y")
                    nc.vector.tensor_add(te[:, 0:n], m0, m1)
                    nc.vector.tensor_add(yt[:, 0:n], te[:, 0:n], m2)
                    nc.vector.tensor_sub(to[:, 0:n], m1, m2)
                    nc.vector.tensor_sub(yt[:, n : 2 * n], to[:, 0:n], m3)
                    # Store on the (otherwise idle) Sync ring.
                    nc.sync.dma_start(
                        out=Od[b, :, :, hp0 : hp0 + nh, :],
                        in_=yt[:, 0 : 2 * n].rearrange(
                            "c (a h j) -> c a h j", a=2, h=nh
                        ),
                    )

            st = emit_load(0)
            for b in range(BL):
                nxt = emit_load(b + 1) if b + 1 < BL else None
                emit_compute(b, st)
                st = nxt

    _split_excess_waits(nc)
    return nc


_CACHE = {}


def _get_nc():
    if "nc" not in _CACHE:
        _CACHE["nc"] = _build()
    return _CACHE["nc"]


def _prep(X, kern):
    """Host-side Winograd F(2,3) input/kernel transforms (bf16)."""
    bf16 = ml_dtypes.bfloat16
    xe = X[:, :, 0::2, :]
    xo = X[:, :, 1::2, :]
    V = np.empty((B, XI, H, NJ, C), np.float32)
    V[:, 0] = xe[:, :, :NJ] - xe[:, :, 1 : NJ + 1]
    V[:, 1] = xo[:, :, :NJ] + xe[:, :, 1 : NJ + 1]
    V[:, 2] = xe[:, :, 1 : NJ + 1] - xo[:, :, :NJ]
    V[:, 3] = xo[:, :, :NJ] - xo[:, :, 1 : NJ + 1]
    Vb = np.ascontiguousarray(V.transpose(0, 4, 2, 1, 3)).astype(bf16)
    G = np.array(
        [[1, 0, 0], [0.5, 0.5, 0.5], [0.5, -0.5, 0.5], [0, 0, 1]], np.float32
    )
    # U[b, kh, xi, ci, co]; t = kh*4+xi
    U = np.einsum("xw,bhwio->bhxio", G, kern)
    Ub = np.ascontiguousarray(U.reshape(B, KK * XI, C, C)).astype(bf16)
    return Vb, Ub


def _run(X, kern, **kw):
    Vb, Ub = _prep(X, kern)
    in_maps = [
        {
            "V": np.ascontiguousarray(Vb[c * BL : (c + 1) * BL]),
            "U": np.ascontiguousarray(Ub[c * BL : (c + 1) * BL]),
        }
        for c in range(N_CORES)
    ]
    last_err = None
    for _attempt in range(3):
        try:
            res = run_bass_kernel_spmd(
                _get_nc(), in_maps, list(range(N_CORES)), **kw
            )
            break
        except Exception as e:  # transient NRT_EXEC_UNIT_UNRECOVERABLE etc.
            last_err = e
    else:
        raise last_err
    # device out: [BL, C, 2, HO, NJ] -> [B, HO, W, C]
    O = np.concatenate(
        [np.asarray(res.results[c]["out"]) for c in range(N_CORES)], axis=0
    ).astype(np.float32)
    out = np.ascontiguousarray(
        O.transpose(0, 3, 4, 2, 1).reshape(B, HO, WO, C)
    )
    return out, res


def kernel(X, kernel):
    X = np.ascontiguousarray(X, dtype=np.float32)
    kern = np.ascontiguousarray(kernel, dtype=np.float32)
    out, _ = _run(X, kern)
    return out


# revision 18
# speedup vs baseline: 1.1671x; 1.1671x over previous
"""Per-sample dynamic conv2d (VALID) on 8 Trainium2 NeuronCores — v4.

v4 = 1-D Winograd F(2,3) along W, direct 3-tap accumulation along H.
Cuts the PE moving-column count 1.5x vs the direct v3 kernel (145k ->
97k streamed columns per sample).

Math per sample (hp = output row, jt = W-tile, w = 2*jt + a):
  V0 = x[2j]-x[2j+2], V1 = x[2j+1]+x[2j+2],
  V2 = x[2j+2]-x[2j+1], V3 = x[2j+1]-x[2j+3]          (host, bf16)
  U[kh,xi] = G @ K[kh,:]  (G = F(2,3) kernel transform) (host, bf16)
  M[xi][hp] = sum_kh V[xi][hp+kh] @ U[kh,xi]            (PE, 12 MMs/group)
  y_even = M0+M1+M2,  y_odd = M1-M2-M3                  (ACT copies + DVE adds)

Device layout: psum/output partitions = Cout.  Output is written as
[C, 2, HO, 63] (even/odd de-interleaved) and the host transposes back —
all device DMA is therefore fully linear.

Per 8-row group x 4 xi: 3 accumulating matmuls (stationary U[kh,xi],
moving = contiguous 504-col V slice) into 4 psum banks; ACT evacuates
M0..M2 to bf16 SBUF, DVE evacuates M3 and does the 4 inverse-transform
adds.  2 groups of psum (8 banks) in flight.
"""

import numpy as np
import ml_dtypes

import concourse.bass as bass
import concourse.mybir as mybir
from concourse.bass_utils import run_bass_kernel_spmd
from concourse.tile import TileContext

N_CORES = 8
B, H, W, C = 32, 128, 128, 128
KK = 3
XI = 4                       # Winograd phases
BL = B // N_CORES            # samples per core
HO = WO = H - KK + 1         # 126
NJ = WO // 2                 # 63 W-tiles
HPG = 8                      # output rows per group
NG = (HO + HPG - 1) // HPG   # 16 groups (last holds 6 rows)
VSZ = XI * H * NJ            # 32256 free elems of V per sample

F32 = mybir.dt.float32
BF16 = mybir.dt.bfloat16


def _split_excess_waits(nc, limit=1):
    """walrus codegen rejects >1 sync-wait on several instruction kinds.
    Move excess waits onto preceding same-engine NoOps."""
    n = 0
    for bb in nc.m.functions[0].blocks:
        out = []
        changed = False
        for inst in bb.instructions:
            si = inst.sync_info
            if si is not None and len(si.on_wait) > limit:
                waits = list(si.on_wait)
                excess, keep = waits[:-limit], waits[-limit:]
                for i in range(0, len(excess), limit):
                    n += 1
                    out.append(
                        mybir.InstNoOp(
                            name=f"I-waitsplit-{n}",
                            engine=inst.engine,
                            bass_nofuse=True,
                            sync_info=mybir.SyncInfo(
                                on_wait=excess[i : i + limit], on_update=[]
                            ),
                        )
                    )
                inst.sync_info = mybir.SyncInfo(on_wait=keep, on_update=si.on_update)
                changed = True
            out.append(inst)
        if changed:
            bb.instructions = out
    return n


RT = 4                       # V row-tiles per sample
RTR = 34                     # rows per V tile (32 + 2 overlap; last tile 32)


def _build():
    nc = bass.Bass()
    # V pre-tiled into 4 overlapping 34-row blocks (xi-major inside each
    # block): early blocks arrive first -> group 0 starts ~6us in, and
    # matmul moving slices stay fully contiguous.
    Vd = nc.declare_dram_parameter(
        "V", [BL, C, RT, XI, RTR, NJ], BF16, isOutput=False
    )
    # t = kh*4 + xi
    Ud = nc.declare_dram_parameter("U", [BL, KK * XI, C, C], BF16, isOutput=False)
    Od = nc.declare_dram_parameter("out", [BL, C, 2, HO, NJ], BF16, isOutput=True)

    with TileContext(nc) as tc:
        with (
            tc.tile_pool(name="vt", bufs=8) as p_v,
            tc.tile_pool(name="ut", bufs=2) as p_u,
            tc.tile_pool(name="mt", bufs=12) as p_m,
            tc.tile_pool(name="tt", bufs=6) as p_t,
            tc.tile_pool(name="yt", bufs=6) as p_y,
            tc.tile_pool(name="pacc", bufs=4, space="PSUM") as p_acc,
        ):
            def emit_load(b):
                vts = []
                for k in range(RT):
                    vt = p_v.tile(
                        [C, XI * RTR * NJ], BF16, tag="vt", name=f"vt{b}_{k}"
                    )
                    nc.sync.dma_start(
                        out=vt[:, :],
                        in_=Vd[b, :, k].rearrange("c x r j -> c (x r j)"),
                    )
                    vts.append(vt)
                ut = p_u.tile([C, KK * XI * C], BF16, tag="ut")
                nc.gpsimd.dma_start(
                    out=ut[:, :].rearrange("ci (t co) -> ci t co", t=KK * XI),
                    in_=Ud[b].rearrange("t ci co -> ci t co"),
                )
                return (vts, ut)

            def emit_compute(b, st):
                vts, ut = st
                for g in range(NG):
                    hp0 = HPG * g
                    nh = min(HPG, HO - hp0)
                    n = nh * NJ
                    vt = vts[g // 4]
                    lr0 = hp0 - 32 * (g // 4)  # local row of hp0 in its tile
                    # Pair psum banks: P01 holds M0 at [:,0:n] (bank A) and
                    # M1 at [:,512:512+n] (bank B); P23 likewise. ACT then
                    # evacuates each pair in ONE activate (FD=512+n).
                    P01 = p_acc.tile([C, 1024], F32, tag="P", name=f"P01_{g}")
                    P23 = p_acc.tile([C, 1024], F32, tag="P", name=f"P23_{g}")
                    for xi in range(XI):
                        pt = P01 if xi < 2 else P23
                        o0 = (xi % 2) * 512
                        for kh in range(KK):
                            off = (xi * RTR + lr0 + kh) * NJ
                            nc.tensor.matmul(
                                pt[:, o0 : o0 + n],
                                ut[:, (kh * XI + xi) * C : (kh * XI + xi + 1) * C],
                                vt[:, off : off + n],
                                start=(kh == 0),
                                stop=(kh == KK - 1),
                            )
                    # ACT evacuates both psum pairs (bf16); DVE does only the
                    # inverse-transform adds (bf16 SBUF, 2x DVE mode).
                    m01 = p_m.tile([C, 1024], BF16, tag="m", name=f"m01_{g}")
                    m23 = p_m.tile([C, 1024], BF16, tag="m", name=f"m23_{g}")
                    nc.scalar.copy(m01[:, 0 : 512 + n], P01[:, 0 : 512 + n])
                    nc.scalar.copy(m23[:, 0 : 512 + n], P23[:, 0 : 512 + n])
                    m0, m1 = m01[:, 0:n], m01[:, 512 : 512 + n]
                    m2, m3 = m23[:, 0:n], m23[:, 512 : 512 + n]
                    te = p_t.tile([C, 504], BF16, tag="t")
                    to = p_t.tile([C, 504], BF16, tag="t")
                    yt = p_y.tile([C, 2 * 504], BF16, tag="y")
                    nc.vector.tensor_add(te[:, 0:n], m0, m1)
                    nc.vector.tensor_add(yt[:, 0:n], te[:, 0:n], m2)
                    nc.vector.tensor_sub(to[:, 0:n], m1, m2)
                    nc.vector.tensor_sub(yt[:, n : 2 * n], to[:, 0:n], m3)
                    # Store on the (otherwise idle) Sync ring.
                    nc.sync.dma_start(
                        out=Od[b, :, :, hp0 : hp0 + nh, :],
                        in_=yt[:, 0 : 2 * n].rearrange(
                            "c (a h j) -> c a h j", a=2, h=nh
                        ),
                    )

            st = emit_load(0)
            for b in range(BL):
                nxt = emit_load(b + 1) if b + 1 < BL else None
                emit_compute(b, st)
                st = nxt

    _split_excess_waits(nc)
    return nc


_CACHE = {}


def _get_nc():
    if "nc" not in _CACHE:
        _CACHE["nc"] = _build()
    return _CACHE["nc"]


def _prep(X, kern):
    """Host-side Winograd F(2,3) input/kernel transforms (bf16)."""
    bf16 = ml_dtypes.bfloat16
    xe = X[:, :, 0::2, :]
    xo = X[:, :, 1::2, :]
    V = np.empty((B, XI, H, NJ, C), np.float32)
    V[:, 0] = xe[:, :, :NJ] - xe[:, :, 1 : NJ + 1]
    V[:, 1] = xo[:, :, :NJ] + xe[:, :, 1 : NJ + 1]
    V[:, 2] = xe[:, :, 1 : NJ + 1] - xo[:, :, :NJ]
    V[:, 3] = xo[:, :, :NJ] - xo[:, :, 1 : NJ + 1]
    # tile into 4 overlapping 34-row blocks, xi-major inside each block:
    # Vt[b, c, k, xi, r, j] = V[b, xi, 32k+r, j, c]
    Vp = np.zeros((B, XI, RT * 32 + 2, NJ, C), np.float32)
    Vp[:, :, :H] = V
    Vt = np.stack(
        [Vp[:, :, 32 * k : 32 * k + RTR] for k in range(RT)], axis=1
    )  # [B, k, xi, r, j, c]
    Vb = np.ascontiguousarray(Vt.transpose(0, 5, 1, 2, 3, 4)).astype(bf16)
    G = np.array(
        [[1, 0, 0], [0.5, 0.5, 0.5], [0.5, -0.5, 0.5], [0, 0, 1]], np.float32
    )
    # U[b, kh, xi, ci, co]; t = kh*4+xi
    U = np.einsum("xw,bhwio->bhxio", G, kern)
    Ub = np.ascontiguousarray(U.reshape(B, KK * XI, C, C)).astype(bf16)
    return Vb, Ub


def _run(X, kern, **kw):
    Vb, Ub = _prep(X, kern)
    in_maps = [
        {
            "V": np.ascontiguousarray(Vb[c * BL : (c + 1) * BL]),
            "U": np.ascontiguousarray(Ub[c * BL : (c + 1) * BL]),
        }
        for c in range(N_CORES)
    ]
    last_err = None
    for _attempt in range(3):
        try:
            res = run_bass_kernel_spmd(
                _get_nc(), in_maps, list(range(N_CORES)), **kw
            )
            break
        except Exception as e:  # transient NRT_EXEC_UNIT_UNRECOVERABLE etc.
            last_err = e
    else:
        raise last_err
    # device out: [BL, C, 2, HO, NJ] -> [B, HO, W, C]
    O = np.concatenate(
        [np.asarray(res.results[c]["out"]) for c in range(N_CORES)], axis=0
    ).astype(np.float32)
    out = np.ascontiguousarray(
        O.transpose(0, 3, 4, 2, 1).reshape(B, HO, WO, C)
    )
    return out, res


def kernel(X, kernel):
    X = np.ascontiguousarray(X, dtype=np.float32)
    kern = np.ascontiguousarray(kernel, dtype=np.float32)
    out, _ = _run(X, kern)
    return out


# revision 19
# speedup vs baseline: 1.2323x; 1.0559x over previous
"""Per-sample dynamic conv2d (VALID) on 8 Trainium2 NeuronCores — v4.

v4 = 1-D Winograd F(2,3) along W, direct 3-tap accumulation along H.
Cuts the PE moving-column count 1.5x vs the direct v3 kernel (145k ->
97k streamed columns per sample).

Math per sample (hp = output row, jt = W-tile, w = 2*jt + a):
  V0 = x[2j]-x[2j+2], V1 = x[2j+1]+x[2j+2],
  V2 = x[2j+2]-x[2j+1], V3 = x[2j+1]-x[2j+3]          (host, bf16)
  U[kh,xi] = G @ K[kh,:]  (G = F(2,3) kernel transform) (host, bf16)
  M[xi][hp] = sum_kh V[xi][hp+kh] @ U[kh,xi]            (PE, 12 MMs/group)
  y_even = M0+M1+M2,  y_odd = M1-M2-M3                  (ACT copies + DVE adds)

Device layout: psum/output partitions = Cout.  Output is written as
[C, 2, HO, 63] (even/odd de-interleaved) and the host transposes back —
all device DMA is therefore fully linear.

Per 8-row group x 4 xi: 3 accumulating matmuls (stationary U[kh,xi],
moving = contiguous 504-col V slice) into 4 psum banks; ACT evacuates
M0..M2 to bf16 SBUF, DVE evacuates M3 and does the 4 inverse-transform
adds.  2 groups of psum (8 banks) in flight.
"""

import numpy as np
import ml_dtypes

import concourse.bass as bass
import concourse.mybir as mybir
from concourse.bass_utils import run_bass_kernel_spmd
from concourse.tile import TileContext

N_CORES = 8
B, H, W, C = 32, 128, 128, 128
KK = 3
XI = 4                       # Winograd phases
BL = B // N_CORES            # samples per core
HO = WO = H - KK + 1         # 126
NJ = WO // 2                 # 63 W-tiles
HPG = 8                      # output rows per group
NG = (HO + HPG - 1) // HPG   # 16 groups (last holds 6 rows)
VSZ = XI * H * NJ            # 32256 free elems of V per sample

F32 = mybir.dt.float32
BF16 = mybir.dt.bfloat16


def _split_excess_waits(nc, limit=1):
    """walrus codegen rejects >1 sync-wait on several instruction kinds.
    Move excess waits onto preceding same-engine NoOps."""
    n = 0
    for bb in nc.m.functions[0].blocks:
        out = []
        changed = False
        for inst in bb.instructions:
            si = inst.sync_info
            if si is not None and len(si.on_wait) > limit:
                waits = list(si.on_wait)
                excess, keep = waits[:-limit], waits[-limit:]
                for i in range(0, len(excess), limit):
                    n += 1
                    out.append(
                        mybir.InstNoOp(
                            name=f"I-waitsplit-{n}",
                            engine=inst.engine,
                            bass_nofuse=True,
                            sync_info=mybir.SyncInfo(
                                on_wait=excess[i : i + limit], on_update=[]
                            ),
                        )
                    )
                inst.sync_info = mybir.SyncInfo(on_wait=keep, on_update=si.on_update)
                changed = True
            out.append(inst)
        if changed:
            bb.instructions = out
    return n


RT = 4                       # V row-tiles per sample
RTR = 34                     # rows per V tile (32 + 2 overlap; last tile 32)


def _build():
    nc = bass.Bass()
    # V pre-tiled into 4 overlapping 34-row blocks (xi-major inside each
    # block): early blocks arrive first -> group 0 starts ~6us in, and
    # matmul moving slices stay fully contiguous.
    Vd = nc.declare_dram_parameter(
        "V", [BL, C, RT, XI, RTR, NJ], BF16, isOutput=False
    )
    # t = kh*4 + xi
    Ud = nc.declare_dram_parameter("U", [BL, KK * XI, C, C], BF16, isOutput=False)
    Od = nc.declare_dram_parameter("out", [BL, C, 2, HO, NJ], BF16, isOutput=True)

    with TileContext(nc) as tc:
        with (
            tc.tile_pool(name="vt", bufs=8) as p_v,
            tc.tile_pool(name="ut", bufs=2) as p_u,
            tc.tile_pool(name="mt", bufs=12) as p_m,
            tc.tile_pool(name="tt", bufs=6) as p_t,
            tc.tile_pool(name="yt", bufs=6) as p_y,
            tc.tile_pool(name="pacc", bufs=4, space="PSUM") as p_acc,
        ):
            def emit_load(b):
                ut = p_u.tile([C, KK * XI * C], BF16, tag="ut")
                # sample 0: U on the fast sync HWDGE ring, before V, so the
                # first matmul isn't gated on a slow SWDGE transfer
                ueng = nc.sync if b == 0 else nc.gpsimd
                ueng.dma_start(
                    out=ut[:, :].rearrange("ci (t co) -> ci t co", t=KK * XI),
                    in_=Ud[b].rearrange("t ci co -> ci t co"),
                )
                vts = []
                TSZ = XI * RTR * NJ
                nchunks = 4 if b == 0 else 1
                step = TSZ // nchunks
                for k in range(RT):
                    vt = p_v.tile([C, TSZ], BF16, tag="vt", name=f"vt{b}_{k}")
                    src = Vd[b, :, k].rearrange("c x r j -> c (x r j)")
                    for c0 in range(0, TSZ, step):
                        nc.sync.dma_start(
                            out=vt[:, c0 : c0 + step],
                            in_=src[:, c0 : c0 + step],
                        )
                    vts.append(vt)
                return (vts, ut)

            def emit_compute(b, st):
                vts, ut = st
                for g in range(NG):
                    hp0 = HPG * g
                    nh = min(HPG, HO - hp0)
                    n = nh * NJ
                    vt = vts[g // 4]
                    lr0 = hp0 - 32 * (g // 4)  # local row of hp0 in its tile
                    # Pair psum banks: P01 holds M0 at [:,0:n] (bank A) and
                    # M1 at [:,512:512+n] (bank B); P23 likewise. ACT then
                    # evacuates each pair in ONE activate (FD=512+n).
                    P01 = p_acc.tile([C, 1024], F32, tag="P", name=f"P01_{g}")
                    P23 = p_acc.tile([C, 1024], F32, tag="P", name=f"P23_{g}")
                    for xi in range(XI):
                        pt = P01 if xi < 2 else P23
                        o0 = (xi % 2) * 512
                        for kh in range(KK):
                            off = (xi * RTR + lr0 + kh) * NJ
                            nc.tensor.matmul(
                                pt[:, o0 : o0 + n],
                                ut[:, (kh * XI + xi) * C : (kh * XI + xi + 1) * C],
                                vt[:, off : off + n],
                                start=(kh == 0),
                                stop=(kh == KK - 1),
                            )
                    # ACT evacuates both psum pairs (bf16); DVE does only the
                    # inverse-transform adds (bf16 SBUF, 2x DVE mode).
                    m01 = p_m.tile([C, 1024], BF16, tag="m", name=f"m01_{g}")
                    m23 = p_m.tile([C, 1024], BF16, tag="m", name=f"m23_{g}")
                    nc.scalar.copy(m01[:, 0 : 512 + n], P01[:, 0 : 512 + n])
                    nc.scalar.copy(m23[:, 0 : 512 + n], P23[:, 0 : 512 + n])
                    m0, m1 = m01[:, 0:n], m01[:, 512 : 512 + n]
                    m2, m3 = m23[:, 0:n], m23[:, 512 : 512 + n]
                    te = p_t.tile([C, 504], BF16, tag="t")
                    to = p_t.tile([C, 504], BF16, tag="t")
                    yt = p_y.tile([C, 2 * 504], BF16, tag="y")
                    nc.vector.tensor_add(te[:, 0:n], m0, m1)
                    nc.vector.tensor_add(yt[:, 0:n], te[:, 0:n], m2)
                    nc.vector.tensor_sub(to[:, 0:n], m1, m2)
                    nc.vector.tensor_sub(yt[:, n : 2 * n], to[:, 0:n], m3)
                    # Store on the (otherwise idle) Sync ring.
                    nc.sync.dma_start(
                        out=Od[b, :, :, hp0 : hp0 + nh, :],
                        in_=yt[:, 0 : 2 * n].rearrange(
                            "c (a h j) -> c a h j", a=2, h=nh
                        ),
                    )

            st = emit_load(0)
            for b in range(BL):
                nxt = emit_load(b + 1) if b + 1 < BL else None
                emit_compute(b, st)
                st = nxt

    _split_excess_waits(nc)
    return nc


_CACHE = {}


def _get_nc():
    if "nc" not in _CACHE:
        _CACHE["nc"] = _build()
    return _CACHE["nc"]


def _prep(X, kern):
    """Host-side Winograd F(2,3) input/kernel transforms (bf16)."""
    bf16 = ml_dtypes.bfloat16
    xe = X[:, :, 0::2, :]
    xo = X[:, :, 1::2, :]
    V = np.empty((B, XI, H, NJ, C), np.float32)
    V[:, 0] = xe[:, :, :NJ] - xe[:, :, 1 : NJ + 1]
    V[:, 1] = xo[:, :, :NJ] + xe[:, :, 1 : NJ + 1]
    V[:, 2] = xe[:, :, 1 : NJ + 1] - xo[:, :, :NJ]
    V[:, 3] = xo[:, :, :NJ] - xo[:, :, 1 : NJ + 1]
    # tile into 4 overlapping 34-row blocks, xi-major inside each block:
    # Vt[b, c, k, xi, r, j] = V[b, xi, 32k+r, j, c]
    Vp = np.zeros((B, XI, RT * 32 + 2, NJ, C), np.float32)
    Vp[:, :, :H] = V
    Vt = np.stack(
        [Vp[:, :, 32 * k : 32 * k + RTR] for k in range(RT)], axis=1
    )  # [B, k, xi, r, j, c]
    Vb = np.ascontiguousarray(Vt.transpose(0, 5, 1, 2, 3, 4)).astype(bf16)
    G = np.array(
        [[1, 0, 0], [0.5, 0.5, 0.5], [0.5, -0.5, 0.5], [0, 0, 1]], np.float32
    )
    # U[b, kh, xi, ci, co]; t = kh*4+xi
    U = np.einsum("xw,bhwio->bhxio", G, kern)
    Ub = np.ascontiguousarray(U.reshape(B, KK * XI, C, C)).astype(bf16)
    return Vb, Ub


def _run(X, kern, **kw):
    Vb, Ub = _prep(X, kern)
    in_maps = [
        {
            "V": np.ascontiguousarray(Vb[c * BL : (c + 1) * BL]),
            "U": np.ascontiguousarray(Ub[c * BL : (c + 1) * BL]),
        }
        for c in range(N_CORES)
    ]
    last_err = None
    for _attempt in range(3):
        try:
            res = run_bass_kernel_spmd(
                _get_nc(), in_maps, list(range(N_CORES)), **kw
            )
            break
        except Exception as e:  # transient NRT_EXEC_UNIT_UNRECOVERABLE etc.
            last_err = e
    else:
        raise last_err
    # device out: [BL, C, 2, HO, NJ] -> [B, HO, W, C]
    O = np.concatenate(
        [np.asarray(res.results[c]["out"]) for c in range(N_CORES)], axis=0
    ).astype(np.float32)
    out = np.ascontiguousarray(
        O.transpose(0, 3, 4, 2, 1).reshape(B, HO, WO, C)
    )
    return out, res


def kernel(X, kernel):
    X = np.ascontiguousarray(X, dtype=np.float32)
    kern = np.ascontiguousarray(kernel, dtype=np.float32)
    out, _ = _run(X, kern)
    return out
